# revision 39
# baseline (speedup 1.0000x reference)
"""Trainium2 Bass kernel for nn_LogicGatedSpikingSelfAttention.

Sharding: channel/head-parallel over 8 cores. Each core owns 128 output
channels = 2 heads for the q/k/v branches (BN stats fully local, since
stats are per-channel over all tokens), runs attention for its 2 heads
over all 4 batches locally, and computes a 128-output-channel slice of
the projection. One AllGather moves the binary attention spikes (+ per-
head energies for the logic gate) between the attention and projection
stages; the gate is folded into the projection weights after the gather
(exact: gate is {0,1}).

Attention uses associativity — there is no softmax, so
x_attn = scale*gate * q @ (k^T @ v). k^T@v is a 64x64 integer count
matrix per (batch, head) (exact in fp16, counts <= 1024), and
S = (k^T v)^T q gives channel-major integer scores identical to the
naive q@k^T@v order. The attn-LIF threshold reduces to S >= 2^0.75.
k/v spikes are transposed to token-major via the DMA XBAR (off the PE).
Per-head small matmuls are packed into PE quadrants via tile_position.
The spike payload crosses cores as fp8e4 ({0,1} exact), energies ride
along as bitcast f32 bytes.
"""
import numpy as np
import ml_dtypes

import concourse.bass as bass
import concourse.bass_isa as bass_isa
import concourse.bacc as bacc
import concourse.tile as tile
from concourse import mybir
from concourse.bass_utils import run_bass_kernel_spmd

NCORES = 8
B, NSEQ, D, H = 4, 1024, 1024, 16
HD = D // H            # 64 head dim
CH = D // NCORES       # 128 channels per core
TOK = B * NSEQ         # 4096 tokens
KT = D // 128          # 8 contraction tiles
NBLK = TOK // 128      # 32 token blocks of 128
EPS = 1e-5
S_TH = float(2.0 ** 0.75)   # x_attn >= 1  <=>  S >= hd**0.125 = 2^0.75
SPIKE_N = 128 * TOK         # flat fp8 payload: spikes then 32B f32 energies
PAYLEN = SPIKE_N + 32
F32 = mybir.dt.float32
BF16 = mybir.dt.bfloat16
FP16 = mybir.dt.float16
FP8 = mybir.dt.float8e4
BF = ml_dtypes.bfloat16

_CACHE = {}


def _build():
    nc = bacc.Bacc("TRN2", target_bir_lowering=False, debug=False,
                   num_devices=NCORES)
    inp = {}
    def din(name, shape, dt=BF16):
        inp[name] = nc.dram_tensor(name, shape, dt, kind="ExternalInput")
        return inp[name]

    din("xT",  [128, KT * TOK])          # host pre-tiled: [p, (t n)]
    din("wq",  [128, KT * CH]); din("wk", [128, KT * CH])
    din("wv",  [128, KT * CH]); din("wp", [128, KT * CH])
    for nm in ("tq", "tk", "tv", "tp"):
        din(nm, [CH, 1], F32)
    din("wgr", [H, H], F32)              # lhsT: [h, h'] = sum_r Wg[h', h+16r]/1024
    din("bgr", [H, 1], F32)
    din("i2e", [CH, 2], F32)             # [p, j] = (p//64==j)
    din("i16", [H, KT * 128], F32)       # [h, (t m)] = (t*128+m)//64 == h
    outT = nc.dram_tensor("outT", [CH, TOK], BF16, kind="ExternalOutput")

    with tile.TileContext(nc) as tc:
        with tc.tile_pool(name="consts", bufs=1) as consts, \
             tc.tile_pool(name="spikes", bufs=1) as spk_pool, \
             tc.tile_pool(name="dram", bufs=1, space="DRAM") as dram:
            _body(tc, inp, outT, consts, spk_pool, dram)
    nc.compile()
    return nc


def _body(tc, inp, outT, consts, spk_pool, dram):
    nc = tc.nc
    V, SC, GP, TE = nc.vector, nc.scalar, nc.gpsimd, nc.tensor
    AF = mybir.ActivationFunctionType
    OP = mybir.AluOpType
    DENG = [nc.sync, nc.scalar, nc.gpsimd]

    # ---- loads ordered by first consumption so the q branch streams ----
    # sync:   xts0 xts3 xts6 smalls wgr i2e
    # scalar: wq   xts1 xts4 xts7 i16
    # gpsimd: wk   xts2 wv   xts5 wp
    xre = inp["xT"].ap().rearrange("p (t n) -> p t n", t=KT)
    wre = {nm: inp[nm].ap().rearrange("p (t m) -> p t m", t=KT)
           for nm in ("wq", "wk", "wv", "wp")}
    w_sb = {nm: consts.tile([128, KT, CH], BF16, name=f"{nm}_sb")
            for nm in ("wq", "wk", "wv", "wp")}
    xts = [consts.tile([128, TOK], BF16, name=f"xts{kt}")
           for kt in range(KT)]
    nc.scalar.dma_start(w_sb["wq"][:], wre["wq"])
    nc.gpsimd.dma_start(w_sb["wk"][:], wre["wk"])
    nc.sync.dma_start(xts[0][:], xre[:, 0, :])
    nc.scalar.dma_start(xts[1][:], xre[:, 1, :])
    nc.gpsimd.dma_start(xts[2][:], xre[:, 2, :])
    nc.sync.dma_start(xts[3][:], xre[:, 3, :])
    nc.scalar.dma_start(xts[4][:], xre[:, 4, :])
    nc.gpsimd.dma_start(w_sb["wv"][:], wre["wv"])
    nc.sync.dma_start(xts[6][:], xre[:, 6, :])
    nc.scalar.dma_start(xts[7][:], xre[:, 7, :])
    nc.gpsimd.dma_start(xts[5][:], xre[:, 5, :])
    nc.gpsimd.dma_start(w_sb["wp"][:], wre["wp"])
    small = {}
    for nm in ("tq", "tk", "tv", "tp", "bgr"):
        t = consts.tile([inp[nm].shape[0], 1], F32, name=f"{nm}_sb")
        nc.sync.dma_start(t[:], inp[nm].ap())
        small[nm] = t
    wgr_sb = consts.tile([H, H], F32)
    nc.sync.dma_start(wgr_sb[:], inp["wgr"].ap())
    i2e_sb = consts.tile([CH, 2], F32)
    nc.sync.dma_start(i2e_sb[:], inp["i2e"].ap())
    i16_sb = consts.tile([H, KT, 128], F32)
    nc.scalar.dma_start(i16_sb[:],
                        inp["i16"].ap().rearrange("h (t m) -> h t m", t=KT))
    eps_sb = consts.tile([128, 1], F32)
    V.memset(eps_sb[:], EPS)
    th128_sb = consts.tile([128, 1], F32)
    V.memset(th128_sb[:], -128.0 * S_TH)

    # ---- DRAM staging for the two AllGathers ----
    e_d = dram.tile([32], FP8)
    gath_e = dram.tile([NCORES, 32], FP8, addr_space="Shared")
    pay_d = dram.tile([SPIKE_N], FP8)
    gath_d = dram.tile([NCORES, SPIKE_N], FP8, addr_space="Shared")

    # ---- persistent spike tensors ----
    spA = {nm: spk_pool.tile([128, TOK], FP16, name=f"sp{nm}A")
           for nm in ("q", "k", "v")}
    ktok = spk_pool.tile([128, NBLK, 128], FP16)   # [tok, blk, ch]
    vtok = spk_pool.tile([128, NBLK, 128], FP16)
    payload = spk_pool.tile([128, TOK], FP8)       # [64h+d, tok] spikes

    # ================= branches (q, k, v) =================
    # Linear bias cancels inside BatchNorm (the mean absorbs it): no bias
    # add anywhere. PSUM banks drain via fast f32 copies (SC+V split) so
    # the next branch's matmuls start ~2us after the last; BN stats and
    # the spike threshold then run off the SBUF copy.
    with tc.tile_pool(name="ybig", bufs=2) as ybig, \
         tc.tile_pool(name="stps", bufs=2) as stp:
        for nm in ("q", "k", "v"):
            Y = ybig.tile([128, TOK], F32, tag="Y")
            # weight-stationary: kt outer, 8 PSUM banks accumulate
            with tc.tile_pool(name=f"brps_{nm}", bufs=1, space="PSUM") as brps:
                ps = [brps.tile([128, 512], F32, name=f"ps{nm}{i}")
                      for i in range(8)]
                for kt in range(KT):
                    for nck in range(8):
                        TE.matmul(ps[nck][:], w_sb["w" + nm][:, kt, :],
                                  xts[kt][:, nck * 512:(nck + 1) * 512],
                                  start=(kt == 0), stop=(kt == KT - 1))
                for i in range(8):
                    if i % 2:
                        V.tensor_copy(Y[:, i * 512:(i + 1) * 512], ps[i][:])
                    else:
                        SC.activation(Y[:, i * 512:(i + 1) * 512], ps[i][:],
                                      AF.Copy)
            stats = stp.tile([128, 8, 6], F32, tag="stats")
            for i in range(8):
                V.bn_stats(stats[:, i, :], Y[:, i * 512:(i + 1) * 512])
            mv = stp.tile([128, 2], F32, tag="mv")
            V.bn_aggr(mv[:], stats[:])
            std = stp.tile([128, 1], F32, tag="std")
            SC.activation(std[:], mv[:, 1:2], AF.Sqrt, bias=eps_sb[:])
            thr = stp.tile([128, 1], F32, tag="thr")
            V.tensor_tensor(thr[:], std[:], small["t" + nm][:], OP.mult)
            V.tensor_tensor(thr[:], thr[:], mv[:, 0:1], OP.add)
            for b in range(B):
                for j in range(2):
                    i = 2 * b + j
                    V.tensor_scalar(spA[nm][:, i * 512:(i + 1) * 512],
                                    Y[:, i * 512:(i + 1) * 512],
                                    thr[:], None, OP.is_ge)
                # token-major spikes for k/v via DMA XBAR (off the PE)
                if nm == "k":
                    nc.sync.dma_start_transpose(
                        ktok[:, 8 * b:8 * b + 8, :],
                        spA["k"][:, b * NSEQ:(b + 1) * NSEQ])
                elif nm == "v":
                    nc.sync.dma_start_transpose(
                        vtok[:, 8 * b:8 * b + 8, :],
                        spA["v"][:, b * NSEQ:(b + 1) * NSEQ])

            if nm == "k":
                # whole energy path off the PE/PSUM (overlaps the v
                # branch): elementwise + masked partition reduces on
                # gpsimd, then the tiny energy AllGather launches early
                # so gate prep can run during the payload gather
                prod = spk_pool.tile([128, TOK], FP16)
                GP.tensor_tensor(prod[:], spA["q"][:], spA["k"][:], OP.mult)
                ech = spk_pool.tile([128, B], F32)
                V.reduce_sum(ech[:],
                             prod[:].rearrange("p (b n) -> p b n", b=B),
                             axis=mybir.AxisListType.X)
                er = []
                for j in range(2):
                    tmp = spk_pool.tile([128, B], F32, name=f"etmp{j}")
                    GP.tensor_scalar(tmp[:], ech[:], i2e_sb[:, j:j + 1],
                                     None, OP.mult)
                    r = spk_pool.tile([128, B], F32, name=f"ered{j}")
                    GP.partition_all_reduce(r[:], tmp[:], 128,
                                            bass_isa.ReduceOp.add)
                    er.append(r)
                for j in range(2):
                    nc.sync.dma_start(
                        e_d[16 * j:16 * j + 16].rearrange("(p w) -> p w", p=1),
                        er[j][0:1, :].bitcast(FP8))
                GP.collective_compute(
                    "AllGather", OP.bypass,
                    ins=[e_d.opt()], outs=[gath_e.opt()],
                    replica_groups=[list(range(NCORES))])

    # ================= energy head-sum + attention =================
    with tc.tile_pool(name="atps", bufs=1, space="PSUM") as atps, \
         tc.tile_pool(name="s2ps", bufs=2, space="PSUM") as s2ps, \
         tc.tile_pool(name="kvsb", bufs=1) as kvsb:


        # KV[b] = k_tok^T @ v_tok per head, heads packed in PE columns
        kv_ps = [atps.tile([128, HD], F32, name=f"kvps{b}") for b in range(B)]
        kv = kvsb.tile([128, B, HD], FP16)
        for b in range(B):
            for mt in range(8):
                blk = b * 8 + mt
                TE.matmul(kv_ps[b][0:HD, :], ktok[:, blk, 0:HD],
                          vtok[:, blk, 0:HD],
                          start=(mt == 0), stop=(mt == 7),
                          tile_position=(0, 0))
                TE.matmul(kv_ps[b][HD:128, :], ktok[:, blk, HD:128],
                          vtok[:, blk, HD:128],
                          start=(mt == 0), stop=(mt == 7),
                          tile_position=(0, HD))
            if b % 2:
                V.tensor_copy(kv[:, b, :], kv_ps[b][:])
            else:
                SC.activation(kv[:, b, :], kv_ps[b][:], AF.Copy)

        # S^T = KV^T @ q  (channel-major scores), heads packed in quadrants.
        # Scores are integers and S_TH = 2^0.75, so |S - S_TH| >= 0.31: a
        # saturated sigmoid on the ACT engine gives exact {0,1} for half
        # the thresholds while the DVE is_ge does the other half.
        pay_re = pay_d[:].rearrange("(p n) -> p n", p=128)
        for b in range(B):
            for ncn in range(2):
                n0 = b * NSEQ + ncn * 512
                s2 = s2ps.tile([128, 512], F32, tag="s2")
                TE.matmul(s2[0:HD, :], kv[0:HD, b, :],
                          spA["q"][0:HD, n0:n0 + 512],
                          start=True, stop=True, tile_position=(0, 0))
                TE.matmul(s2[HD:128, :], kv[HD:128, b, :],
                          spA["q"][HD:128, n0:n0 + 512],
                          start=True, stop=True, tile_position=(HD, HD))
                if ncn:
                    V.tensor_scalar(payload[:, n0:n0 + 512], s2[:], S_TH,
                                    None, OP.is_ge)
                else:
                    SC.activation(payload[:, n0:n0 + 512], s2[:],
                                  AF.Sigmoid, scale=128.0, bias=th128_sb[:])
            DENG[b % 2].dma_start(
                pay_re[:, b * NSEQ:(b + 1) * NSEQ],
                payload[:, b * NSEQ:(b + 1) * NSEQ])

    # ================= AllGather (flat fp8, contiguous) =================
    GP.collective_compute("AllGather", OP.bypass,
                          ins=[pay_d.opt()], outs=[gath_d.opt()],
                          replica_groups=[list(range(NCORES))])

    # ================= gate -> gated proj weights =================
    with tc.tile_pool(name="gtmp", bufs=1) as gtmp, \
         tc.tile_pool(name="post", bufs=1) as post, \
         tc.tile_pool(name="rhsp", bufs=3) as rhsp, \
         tc.tile_pool(name="pstat", bufs=1) as pstat:
        with tc.tile_pool(name="gtps", bufs=2, space="PSUM") as gtps:
            eg_bytes = gtmp.tile([H, 16], FP8)
            for c in range(NCORES):
                DENG[c % 3].dma_start(
                    eg_bytes[2 * c:2 * c + 2, :],
                    gath_e[c, :].rearrange("(p w) -> p w", p=2))
            g_ps = gtps.tile([H, B], F32, tag="gps")
            TE.matmul(g_ps[:], wgr_sb[:], eg_bytes[:].bitcast(F32),
                      start=True, stop=True)
            gate = gtmp.tile([H, B], F32)
            V.tensor_scalar(gate[:], g_ps[:], small["bgr"][:], 0.5,
                            OP.add, OP.is_ge)
            gv = gtmp.tile([128, KT, B], F32)
            for t in range(KT):
                gv_ps = gtps.tile([128, B], F32, tag="gvps")
                TE.matmul(gv_ps[:], i16_sb[:, t, :], gate[:],
                          start=True, stop=True)
                V.tensor_copy(gv[:, t, :], gv_ps[:])
            wpg = []
            for t in range(KT):
                w = post.tile([128, B, 128], BF16, name=f"wpg{t}")
                for b in range(B):
                    if (t * B + b) % 2:
                        V.tensor_scalar(w[:, b, :], w_sb["wp"][:, t, :],
                                        gv[:, t, b:b + 1], None, OP.mult)
                    else:
                        SC.activation(w[:, b, :], w_sb["wp"][:, t, :],
                                      AF.Identity, scale=gv[:, t, b:b + 1])
                wpg.append(w)

        # ================= projection (fp8 rhs, bf16 weights) ==========
        with tc.tile_pool(name="ppps", bufs=1, space="PSUM") as ppps:
            pp = [ppps.tile([128, 512], F32, name=f"pp{i}") for i in range(8)]
            rhs = []
            for t in range(KT):
                r = rhsp.tile([128, TOK], FP8, tag="rhs")
                DENG[t % 3].dma_start(
                    r[:],
                    gath_d[t, 0:SPIKE_N].rearrange("(p n) -> p n", p=128))
                rhs.append(r)
            for t in range(KT):
                for b in range(B):
                    for ncn in range(2):
                        n0 = b * NSEQ + ncn * 512
                        TE.matmul(pp[b * 2 + ncn][:], wpg[t][:, b, :],
                                  rhs[t][:, n0:n0 + 512],
                                  start=(t == 0), stop=(t == KT - 1))
            # BN stats + spike threshold directly from PSUM (bias cancels)
            stats = pstat.tile([128, 8, 6], F32)
            for i in range(8):
                V.bn_stats(stats[:, i, :], pp[i][:])
            mv = pstat.tile([128, 2], F32)
            V.bn_aggr(mv[:], stats[:])
            std = pstat.tile([128, 1], F32)
            SC.activation(std[:], mv[:, 1:2], AF.Sqrt, bias=eps_sb[:])
            thr = pstat.tile([128, 1], F32)
            V.tensor_tensor(thr[:], std[:], small["tp"][:], OP.mult)
            V.tensor_tensor(thr[:], thr[:], mv[:, 0:1], OP.add)
            osb = pstat.tile([128, TOK], BF16)
            for i in range(8):
                V.tensor_scalar(osb[:, i * 512:(i + 1) * 512], pp[i][:],
                                thr[:], None, OP.is_ge)
                DENG[i % 2].dma_start(
                    outT.ap().rearrange("p (c n) -> p c n", c=8)[:, i, :],
                    osb[:, i * 512:(i + 1) * 512])


def _tile_rows(a):
    # (8*128, N) -> (128, 8*N) so the SBUF [p, (t n)] load is contiguous
    n = a.shape[1]
    return np.ascontiguousarray(
        a.reshape(KT, 128, n).transpose(1, 0, 2).reshape(128, KT * n))


def _prep_inputs(inputs):
    x = np.asarray(inputs["x"], np.float32)
    xT = _tile_rows(x.reshape(TOK, D).T.astype(BF))
    Wg = np.asarray(inputs["Wg"], np.float64)
    wgr = (Wg.reshape(H, HD, H).sum(axis=1).T / 1024.0).astype(np.float32)
    wgr = np.ascontiguousarray(wgr)                     # [h, h']
    bgr = np.asarray(inputs["bg"], np.float32).reshape(H, 1)
    i2e = np.zeros((CH, 2), np.float32)
    i2e[0:HD, 0] = 1.0
    i2e[HD:CH, 1] = 1.0
    i16 = np.zeros((H, D), np.float32)
    for h in range(H):
        i16[h, h * HD:(h + 1) * HD] = 1.0
    i16 = np.ascontiguousarray(
        i16.reshape(H, KT, 128).reshape(H, KT * 128))
    in_maps = []
    for c in range(NCORES):
        sl = slice(CH * c, CH * c + CH)
        m = {"xT": xT, "wgr": wgr, "bgr": bgr, "i2e": i2e, "i16": i16}
        for nm in ("q", "k", "v", "p"):
            W = np.asarray(inputs[f"W{nm}"], np.float32)
            m["w" + nm] = _tile_rows(W[sl, :].T.astype(BF))
            g = np.asarray(inputs[f"g{nm}"], np.float32)[sl]
            be = np.asarray(inputs[f"beta{nm}"], np.float32)[sl]
            m["t" + nm] = ((2.0 - be) / g).reshape(CH, 1).astype(np.float32)
        in_maps.append(m)
    return in_maps


def _run(inputs, trace=False):
    if "nc" not in _CACHE:
        _CACHE["nc"] = _build()
    nc = _CACHE["nc"]
    in_maps = _prep_inputs(inputs)
    res = run_bass_kernel_spmd(nc, in_maps, core_ids=list(range(NCORES)),
                               trace=trace)
    out = np.empty((TOK, D), np.float32)
    for c in range(NCORES):
        out[:, CH * c:CH * c + CH] = res.results[c]["outT"].astype(np.float32).T
    return out.reshape(B, NSEQ, D), res


def kernel(**inputs) -> np.ndarray:
    out, _ = _run(inputs, trace=False)
    return out


# revision 42
# speedup vs baseline: 1.0076x; 1.0076x over previous
"""Trainium2 Bass kernel for nn_LogicGatedSpikingSelfAttention.

Sharding: channel/head-parallel over 8 cores. Each core owns 128 output
channels = 2 heads for the q/k/v branches (BN stats fully local, since
stats are per-channel over all tokens), runs attention for its 2 heads
over all 4 batches locally, and computes a 128-output-channel slice of
the projection. One AllGather moves the binary attention spikes (+ per-
head energies for the logic gate) between the attention and projection
stages; the gate is folded into the projection weights after the gather
(exact: gate is {0,1}).

Attention uses associativity — there is no softmax, so
x_attn = scale*gate * q @ (k^T @ v). k^T@v is a 64x64 integer count
matrix per (batch, head) (exact in fp16, counts <= 1024), and
S = (k^T v)^T q gives channel-major integer scores identical to the
naive q@k^T@v order. The attn-LIF threshold reduces to S >= 2^0.75.
k/v spikes are transposed to token-major via the DMA XBAR (off the PE).
Per-head small matmuls are packed into PE quadrants via tile_position.
The spike payload crosses cores as fp8e4 ({0,1} exact), energies ride
along as bitcast f32 bytes.
"""
import numpy as np
import ml_dtypes

import concourse.bass as bass
import concourse.bass_isa as bass_isa
import concourse.bacc as bacc
import concourse.tile as tile
from concourse import mybir
from concourse.bass_utils import run_bass_kernel_spmd

NCORES = 8
B, NSEQ, D, H = 4, 1024, 1024, 16
HD = D // H            # 64 head dim
CH = D // NCORES       # 128 channels per core
TOK = B * NSEQ         # 4096 tokens
KT = D // 128          # 8 contraction tiles
NBLK = TOK // 128      # 32 token blocks of 128
EPS = 1e-5
S_TH = float(2.0 ** 0.75)   # x_attn >= 1  <=>  S >= hd**0.125 = 2^0.75
SPIKE_N = 128 * TOK         # flat fp8 payload: spikes then 32B f32 energies
PAYLEN = SPIKE_N + 32
F32 = mybir.dt.float32
BF16 = mybir.dt.bfloat16
FP16 = mybir.dt.float16
FP8 = mybir.dt.float8e4
BF = ml_dtypes.bfloat16

_CACHE = {}


def _build():
    nc = bacc.Bacc("TRN2", target_bir_lowering=False, debug=False,
                   num_devices=NCORES)
    inp = {}
    def din(name, shape, dt=BF16):
        inp[name] = nc.dram_tensor(name, shape, dt, kind="ExternalInput")
        return inp[name]

    din("xT",  [128, KT * TOK])          # host pre-tiled: [p, (t n)]
    din("wq",  [128, KT * CH]); din("wk", [128, KT * CH])
    din("wv",  [128, KT * CH]); din("wp", [128, KT * CH])
    for nm in ("tq", "tk", "tv", "tp"):
        din(nm, [CH, 1], F32)
    din("wgr", [H, H], F32)              # lhsT: [h, h'] = sum_r Wg[h', h+16r]/1024
    din("bgr", [H, 1], F32)
    din("i2e", [CH, 2], F32)             # [p, j] = (p//64==j)
    din("i16", [H, KT * 128], F32)       # [h, (t m)] = (t*128+m)//64 == h
    outT = nc.dram_tensor("outT", [CH, TOK], BF16, kind="ExternalOutput")

    with tile.TileContext(nc) as tc:
        with tc.tile_pool(name="consts", bufs=1) as consts, \
             tc.tile_pool(name="spikes", bufs=1) as spk_pool, \
             tc.tile_pool(name="dram", bufs=1, space="DRAM") as dram:
            _body(tc, inp, outT, consts, spk_pool, dram)
    nc.compile()
    return nc


def _body(tc, inp, outT, consts, spk_pool, dram):
    nc = tc.nc
    V, SC, GP, TE = nc.vector, nc.scalar, nc.gpsimd, nc.tensor
    AF = mybir.ActivationFunctionType
    OP = mybir.AluOpType
    DENG = [nc.sync, nc.scalar, nc.gpsimd]

    # ---- loads ordered by first consumption so the q branch streams ----
    # sync:   xts0 xts3 xts6 smalls wgr i2e
    # scalar: wq   xts1 xts4 xts7 i16
    # gpsimd: wk   xts2 wv   xts5 wp
    xre = inp["xT"].ap().rearrange("p (t n) -> p t n", t=KT)
    wre = {nm: inp[nm].ap().rearrange("p (t m) -> p t m", t=KT)
           for nm in ("wq", "wk", "wv", "wp")}
    w_sb = {nm: consts.tile([128, KT, CH], BF16, name=f"{nm}_sb")
            for nm in ("wq", "wk", "wv", "wp")}
    xts = [consts.tile([128, TOK], BF16, name=f"xts{kt}")
           for kt in range(KT)]
    nc.scalar.dma_start(w_sb["wq"][:], wre["wq"])
    nc.gpsimd.dma_start(w_sb["wk"][:], wre["wk"])
    nc.sync.dma_start(xts[0][:], xre[:, 0, :])
    nc.scalar.dma_start(xts[1][:], xre[:, 1, :])
    nc.gpsimd.dma_start(xts[2][:], xre[:, 2, :])
    nc.sync.dma_start(xts[3][:], xre[:, 3, :])
    nc.scalar.dma_start(xts[4][:], xre[:, 4, :])
    nc.gpsimd.dma_start(w_sb["wv"][:], wre["wv"])
    nc.sync.dma_start(xts[6][:], xre[:, 6, :])
    nc.scalar.dma_start(xts[7][:], xre[:, 7, :])
    nc.gpsimd.dma_start(xts[5][:], xre[:, 5, :])
    nc.gpsimd.dma_start(w_sb["wp"][:], wre["wp"])
    small = {}
    for nm in ("tq", "tk", "tv", "tp", "bgr"):
        t = consts.tile([inp[nm].shape[0], 1], F32, name=f"{nm}_sb")
        nc.sync.dma_start(t[:], inp[nm].ap())
        small[nm] = t
    wgr_sb = consts.tile([H, H], F32)
    nc.sync.dma_start(wgr_sb[:], inp["wgr"].ap())
    i2e_sb = consts.tile([CH, 2], F32)
    nc.sync.dma_start(i2e_sb[:], inp["i2e"].ap())
    i16_sb = consts.tile([H, KT, 128], F32)
    nc.scalar.dma_start(i16_sb[:],
                        inp["i16"].ap().rearrange("h (t m) -> h t m", t=KT))
    eps_sb = consts.tile([128, 1], F32)
    V.memset(eps_sb[:], EPS)
    th128_sb = consts.tile([128, 1], F32)
    V.memset(th128_sb[:], -128.0 * S_TH)

    # ---- DRAM staging for the two AllGathers ----
    e_d = dram.tile([32], FP8)
    gath_e = dram.tile([NCORES, 32], FP8, addr_space="Shared")
    pay_d = dram.tile([SPIKE_N], FP8)
    gath_d = dram.tile([NCORES, SPIKE_N], FP8, addr_space="Shared")

    # ---- persistent spike tensors ----
    e_sb = spk_pool.tile([2, B], F32)
    spA = {nm: spk_pool.tile([128, TOK], FP16, name=f"sp{nm}A")
           for nm in ("q", "k", "v")}
    ktok = spk_pool.tile([128, NBLK, 128], FP16)   # [tok, blk, ch]
    vtok = spk_pool.tile([128, NBLK, 128], FP16)
    payload = spk_pool.tile([128, TOK], FP8)       # [64h+d, tok] spikes

    # ================= branches (q, k, v) =================
    # Linear bias cancels inside BatchNorm (the mean absorbs it): no bias
    # add anywhere. PSUM banks drain via fast f32 copies (SC+V split) so
    # the next branch's matmuls start ~2us after the last; BN stats and
    # the spike threshold then run off the SBUF copy.
    with tc.tile_pool(name="ybig", bufs=2) as ybig, \
         tc.tile_pool(name="stps", bufs=2) as stp:
        for nm in ("q", "k", "v"):
            Y = ybig.tile([128, TOK], F32, tag="Y")
            # weight-stationary: kt outer, 8 PSUM banks accumulate
            with tc.tile_pool(name=f"brps_{nm}", bufs=1, space="PSUM") as brps:
                ps = [brps.tile([128, 512], F32, name=f"ps{nm}{i}")
                      for i in range(8)]
                for kt in range(KT):
                    for nck in range(8):
                        TE.matmul(ps[nck][:], w_sb["w" + nm][:, kt, :],
                                  xts[kt][:, nck * 512:(nck + 1) * 512],
                                  start=(kt == 0), stop=(kt == KT - 1))
                for i in range(8):
                    if i % 2:
                        V.tensor_copy(Y[:, i * 512:(i + 1) * 512], ps[i][:])
                    else:
                        SC.activation(Y[:, i * 512:(i + 1) * 512], ps[i][:],
                                      AF.Copy)
            stats = stp.tile([128, 8, 6], F32, tag="stats")
            for i in range(8):
                V.bn_stats(stats[:, i, :], Y[:, i * 512:(i + 1) * 512])
            mv = stp.tile([128, 2], F32, tag="mv")
            V.bn_aggr(mv[:], stats[:])
            std = stp.tile([128, 1], F32, tag="std")
            SC.activation(std[:], mv[:, 1:2], AF.Sqrt, bias=eps_sb[:])
            thr = stp.tile([128, 1], F32, tag="thr")
            V.tensor_tensor(thr[:], std[:], small["t" + nm][:], OP.mult)
            V.tensor_tensor(thr[:], thr[:], mv[:, 0:1], OP.add)
            for b in range(B):
                for j in range(2):
                    i = 2 * b + j
                    V.tensor_scalar(spA[nm][:, i * 512:(i + 1) * 512],
                                    Y[:, i * 512:(i + 1) * 512],
                                    thr[:], None, OP.is_ge)
                # token-major spikes for k/v via DMA XBAR (off the PE)
                if nm == "k":
                    nc.sync.dma_start_transpose(
                        ktok[:, 8 * b:8 * b + 8, :],
                        spA["k"][:, b * NSEQ:(b + 1) * NSEQ])
                elif nm == "v":
                    nc.sync.dma_start_transpose(
                        vtok[:, 8 * b:8 * b + 8, :],
                        spA["v"][:, b * NSEQ:(b + 1) * NSEQ])

            if nm == "k":
                # energy elementwise part on gpsimd (overlaps v branch)
                prod = spk_pool.tile([128, TOK], FP16)
                GP.tensor_tensor(prod[:], spA["q"][:], spA["k"][:], OP.mult)
                ech = spk_pool.tile([128, B], F32)
                V.reduce_sum(ech[:],
                             prod[:].rearrange("p (b n) -> p b n", b=B),
                             axis=mybir.AxisListType.X)

    # ================= energy head-sum + attention =================
    with tc.tile_pool(name="atps", bufs=1, space="PSUM") as atps, \
         tc.tile_pool(name="s2ps", bufs=2, space="PSUM") as s2ps, \
         tc.tile_pool(name="kvsb", bufs=1) as kvsb:
        # tiny energy AllGather first: it overlaps the attention tail and
        # lets the gate/weight-gating run during the payload gather
        e_ps = atps.tile([2, B], F32, name="eps")
        TE.matmul(e_ps[:], i2e_sb[:], ech[:], start=True, stop=True)
        V.tensor_copy(e_sb[:], e_ps[:])
        nc.scalar.dma_start(e_d[:].rearrange("(p w) -> p w", p=2),
                            e_sb[:].bitcast(FP8))
        GP.collective_compute("AllGather", OP.bypass,
                              ins=[e_d.opt()], outs=[gath_e.opt()],
                              replica_groups=[list(range(NCORES))])


        # KV[b] = k_tok^T @ v_tok per head, heads packed in PE columns
        kv_ps = [atps.tile([128, HD], F32, name=f"kvps{b}") for b in range(B)]
        kv = kvsb.tile([128, B, HD], FP16)
        for b in range(B):
            for mt in range(8):
                blk = b * 8 + mt
                TE.matmul(kv_ps[b][0:HD, :], ktok[:, blk, 0:HD],
                          vtok[:, blk, 0:HD],
                          start=(mt == 0), stop=(mt == 7),
                          tile_position=(0, 0))
                TE.matmul(kv_ps[b][HD:128, :], ktok[:, blk, HD:128],
                          vtok[:, blk, HD:128],
                          start=(mt == 0), stop=(mt == 7),
                          tile_position=(0, HD))
            if b % 2:
                V.tensor_copy(kv[:, b, :], kv_ps[b][:])
            else:
                SC.activation(kv[:, b, :], kv_ps[b][:], AF.Copy)

        # S^T = KV^T @ q  (channel-major scores), heads packed in quadrants.
        # Scores are integers and S_TH = 2^0.75, so |S - S_TH| >= 0.31: a
        # saturated sigmoid on the ACT engine gives exact {0,1} for half
        # the thresholds while the DVE is_ge does the other half.
        pay_re = pay_d[:].rearrange("(p n) -> p n", p=128)
        for b in range(B):
            for ncn in range(2):
                n0 = b * NSEQ + ncn * 512
                s2 = s2ps.tile([128, 512], F32, tag="s2")
                TE.matmul(s2[0:HD, :], kv[0:HD, b, :],
                          spA["q"][0:HD, n0:n0 + 512],
                          start=True, stop=True, tile_position=(0, 0))
                TE.matmul(s2[HD:128, :], kv[HD:128, b, :],
                          spA["q"][HD:128, n0:n0 + 512],
                          start=True, stop=True, tile_position=(HD, HD))
                if ncn:
                    V.tensor_scalar(payload[:, n0:n0 + 512], s2[:], S_TH,
                                    None, OP.is_ge)
                else:
                    SC.activation(payload[:, n0:n0 + 512], s2[:],
                                  AF.Sigmoid, scale=128.0, bias=th128_sb[:])
            DENG[b % 2].dma_start(
                pay_re[:, b * NSEQ:(b + 1) * NSEQ],
                payload[:, b * NSEQ:(b + 1) * NSEQ])

    # ================= AllGather (flat fp8, contiguous) =================
    GP.collective_compute("AllGather", OP.bypass,
                          ins=[pay_d.opt()], outs=[gath_d.opt()],
                          replica_groups=[list(range(NCORES))])

    # ================= gate -> gated proj weights =================
    with tc.tile_pool(name="gtmp", bufs=1) as gtmp, \
         tc.tile_pool(name="post", bufs=1) as post, \
         tc.tile_pool(name="rhsp", bufs=3) as rhsp, \
         tc.tile_pool(name="pstat", bufs=1) as pstat:
        with tc.tile_pool(name="gtps", bufs=2, space="PSUM") as gtps:
            eg_bytes = gtmp.tile([H, 16], FP8)
            for c in range(NCORES):
                DENG[c % 3].dma_start(
                    eg_bytes[2 * c:2 * c + 2, :],
                    gath_e[c, :].rearrange("(p w) -> p w", p=2))
            g_ps = gtps.tile([H, B], F32, tag="gps")
            TE.matmul(g_ps[:], wgr_sb[:], eg_bytes[:].bitcast(F32),
                      start=True, stop=True)
            gate = gtmp.tile([H, B], F32)
            V.tensor_scalar(gate[:], g_ps[:], small["bgr"][:], 0.5,
                            OP.add, OP.is_ge)
            gv = gtmp.tile([128, KT, B], F32)
            for t in range(KT):
                gv_ps = gtps.tile([128, B], F32, tag="gvps")
                TE.matmul(gv_ps[:], i16_sb[:, t, :], gate[:],
                          start=True, stop=True)
                V.tensor_copy(gv[:, t, :], gv_ps[:])
            wpg = []
            for t in range(KT):
                w = post.tile([128, B, 128], BF16, name=f"wpg{t}")
                for b in range(B):
                    if (t * B + b) % 2:
                        V.tensor_scalar(w[:, b, :], w_sb["wp"][:, t, :],
                                        gv[:, t, b:b + 1], None, OP.mult)
                    else:
                        SC.activation(w[:, b, :], w_sb["wp"][:, t, :],
                                      AF.Identity, scale=gv[:, t, b:b + 1])
                wpg.append(w)

        # ================= projection (fp8 rhs, bf16 weights) ==========
        with tc.tile_pool(name="ppps", bufs=1, space="PSUM") as ppps:
            pp = [ppps.tile([128, 512], F32, name=f"pp{i}") for i in range(8)]
            rhs = []
            for t in range(KT):
                r = rhsp.tile([128, TOK], FP8, tag="rhs")
                DENG[t % 3].dma_start(
                    r[:],
                    gath_d[t, 0:SPIKE_N].rearrange("(p n) -> p n", p=128))
                rhs.append(r)
            for t in range(KT):
                for b in range(B):
                    for ncn in range(2):
                        n0 = b * NSEQ + ncn * 512
                        TE.matmul(pp[b * 2 + ncn][:], wpg[t][:, b, :],
                                  rhs[t][:, n0:n0 + 512],
                                  start=(t == 0), stop=(t == KT - 1))
            # BN stats + spike threshold directly from PSUM (bias cancels)
            stats = pstat.tile([128, 8, 6], F32)
            for i in range(8):
                V.bn_stats(stats[:, i, :], pp[i][:])
            mv = pstat.tile([128, 2], F32)
            V.bn_aggr(mv[:], stats[:])
            std = pstat.tile([128, 1], F32)
            SC.activation(std[:], mv[:, 1:2], AF.Sqrt, bias=eps_sb[:])
            thr = pstat.tile([128, 1], F32)
            V.tensor_tensor(thr[:], std[:], small["tp"][:], OP.mult)
            V.tensor_tensor(thr[:], thr[:], mv[:, 0:1], OP.add)
            osb = pstat.tile([128, TOK], BF16)
            for i in range(8):
                V.tensor_scalar(osb[:, i * 512:(i + 1) * 512], pp[i][:],
                                thr[:], None, OP.is_ge)
                DENG[i % 2].dma_start(
                    outT.ap().rearrange("p (c n) -> p c n", c=8)[:, i, :],
                    osb[:, i * 512:(i + 1) * 512])


def _tile_rows(a):
    # (8*128, N) -> (128, 8*N) so the SBUF [p, (t n)] load is contiguous
    n = a.shape[1]
    return np.ascontiguousarray(
        a.reshape(KT, 128, n).transpose(1, 0, 2).reshape(128, KT * n))


def _prep_inputs(inputs):
    x = np.asarray(inputs["x"], np.float32)
    xT = _tile_rows(x.reshape(TOK, D).T.astype(BF))
    Wg = np.asarray(inputs["Wg"], np.float64)
    wgr = (Wg.reshape(H, HD, H).sum(axis=1).T / 1024.0).astype(np.float32)
    wgr = np.ascontiguousarray(wgr)                     # [h, h']
    bgr = np.asarray(inputs["bg"], np.float32).reshape(H, 1)
    i2e = np.zeros((CH, 2), np.float32)
    i2e[0:HD, 0] = 1.0
    i2e[HD:CH, 1] = 1.0
    i16 = np.zeros((H, D), np.float32)
    for h in range(H):
        i16[h, h * HD:(h + 1) * HD] = 1.0
    i16 = np.ascontiguousarray(
        i16.reshape(H, KT, 128).reshape(H, KT * 128))
    in_maps = []
    for c in range(NCORES):
        sl = slice(CH * c, CH * c + CH)
        m = {"xT": xT, "wgr": wgr, "bgr": bgr, "i2e": i2e, "i16": i16}
        for nm in ("q", "k", "v", "p"):
            W = np.asarray(inputs[f"W{nm}"], np.float32)
            m["w" + nm] = _tile_rows(W[sl, :].T.astype(BF))
            g = np.asarray(inputs[f"g{nm}"], np.float32)[sl]
            be = np.asarray(inputs[f"beta{nm}"], np.float32)[sl]
            m["t" + nm] = ((2.0 - be) / g).reshape(CH, 1).astype(np.float32)
        in_maps.append(m)
    return in_maps


def _run(inputs, trace=False):
    if "nc" not in _CACHE:
        _CACHE["nc"] = _build()
    nc = _CACHE["nc"]
    in_maps = _prep_inputs(inputs)
    res = run_bass_kernel_spmd(nc, in_maps, core_ids=list(range(NCORES)),
                               trace=trace)
    out = np.empty((TOK, D), np.float32)
    for c in range(NCORES):
        out[:, CH * c:CH * c + CH] = res.results[c]["outT"].astype(np.float32).T
    return out.reshape(B, NSEQ, D), res


def kernel(**inputs) -> np.ndarray:
    out, _ = _run(inputs, trace=False)
    return out
